# revision 8
# baseline (speedup 1.0000x reference)
"""Trainium2 Bass kernel for nn_DynamicGraphEmbedding (adaptive-graph GCN layer).

Computation (matches reference):
  xn[n,b,l] = x[b,l,n]
  x_norm = xn / ||xn||_2 (over l, per (n,b))
  mean_sim = (1/B) sum_b Xn_b Xn_b^T                [N,N]
  top-k neighbor mask per row (k=307 non-self of top-308 incl self)
  A = mean_sim * mask ; deg = A.sum(axis=0) ; dinv = rsqrt(deg) where >0
  An = dinv[s] * A * dinv[d]
  out[d,b,l] = sum_s An[s,d] * (xn_raw @ W)[s,b,l] + bias[l]

Distribution over 8 cores:
  - batch-parallel similarity: each core computes G_partial = sum_{b in shard}
    Xn_b Xn_b^T (G = B*mean_sim; the 1/B cancels in the symmetric
    normalization), ReduceScatter -> each core owns 128 rows of G.
  - per-row top-k threshold by dyadic bisection on the owned rows
    (count(G >= t) >= 308 incl. self).
  - AllGather of masked A rows -> full A everywhere; deg/dinv/An computed
    redundantly (cheap); aggregation is batch-parallel: each core computes
    out^T_b = xw_b^T @ An for its 4 batches and writes its output shard.

All matmuls run as float32r (near-fp32 precision, full PE rate at free>=256).
"""
import sys

if "/opt/trn_rl_repo" not in sys.path:
    sys.path.insert(0, "/opt/trn_rl_repo")

import numpy as np

import concourse.bass as bass
from concourse import bacc
import concourse.mybir as mybir
from concourse.tile import TileContext
from concourse.bass_utils import run_bass_kernel_spmd

B, L, N = 32, 256, 1024
NC = 8
BPC = B // NC          # batches per core
ROWS = N // NC         # owned similarity rows per core
KSEL = max(int(N * 0.3), 1) + 1   # 308: top-k incl. self
NITER = 26             # bisection iterations; resolution 66/2^26 ~ 1e-6
KC = L // 128          # 2 contraction chunks over L
MC = N // 128          # 8 chunks over N
NF = N // 512          # 2 free-dim chunks over N

FP32 = mybir.dt.float32
FP32R = mybir.dt.float32r
AL = mybir.AluOpType

_CACHE = {}


def _build():
    nc = bacc.Bacc(None, target_bir_lowering=False, debug=False)
    x_ext = nc.declare_dram_parameter("x", [BPC, L, N], FP32, isOutput=False)
    w_ext = nc.declare_dram_parameter("w", [L, L], FP32, isOutput=False)
    b_ext = nc.declare_dram_parameter("bias", [1, L], FP32, isOutput=False)
    r_ext = nc.declare_dram_parameter("ridx", [128, 1], FP32, isOutput=False)
    o_ext = nc.declare_dram_parameter("out", [BPC, L, N], FP32, isOutput=True)

    with TileContext(nc) as tc:
        with (
            tc.tile_pool(name="persist", bufs=1) as pp,
            tc.tile_pool(name="big8", bufs=8) as big8,
            tc.tile_pool(name="rot", bufs=3) as rot,
            tc.tile_pool(name="ps4", bufs=4, space="PSUM") as ps4,
            tc.tile_pool(name="ps2", bufs=2, space="PSUM") as ps2,
            tc.tile_pool(name="dram", bufs=1, space="DRAM") as dram,
        ):
            # ---- constants & small inputs ----
            onesc_f = pp.tile([128, 1], FP32, name="onesc_f")
            nc.vector.memset(onesc_f[:], 1.0)
            onesr_f = pp.tile([1, 512], FP32, name="onesr_f")
            nc.vector.memset(onesr_f[:], 1.0)
            ones_col = pp.tile([128, 1], FP32R, name="ones_col")
            nc.vector.tensor_copy(ones_col[:], onesc_f[:])
            ones_row = pp.tile([1, 512], FP32R, name="ones_row")
            nc.vector.tensor_copy(ones_row[:], onesr_f[:])
            one_t = pp.tile([1, 1], FP32R, name="one_t")
            nc.vector.tensor_copy(one_t[:], onesr_f[0:1, 0:1])
            ridx = pp.tile([128, 1], FP32, name="ridx_sb")
            nc.sync.dma_start(ridx[:], r_ext[:])
            bias_sb = pp.tile([1, L], FP32R, name="bias_sb")
            nc.sync.dma_start(bias_sb[:], b_ext.bitcast(FP32R)[:])
            w_sb = []
            for k in range(KC):
                wt = pp.tile([128, L], FP32R, name=f"w_sb{k}")
                nc.sync.dma_start(wt[:],
                                  w_ext[k * 128:(k + 1) * 128, :].bitcast(FP32R))
                w_sb.append(wt)

            # self-exclusion mask: selfm[p, c] = (c != ridx[p])
            iof = pp.tile([128, N], FP32, name="iof")  # reused as bisect scratch
            nc.gpsimd.iota(iof[:], pattern=[[1, N]], base=0, channel_multiplier=0,
                           allow_small_or_imprecise_dtypes=True)
            selfm = pp.tile([128, N], FP32, name="selfm")
            nc.vector.tensor_scalar(selfm[:], iof[:], ridx[:], None, AL.not_equal)

            # ---- phase A: load x, normalize per (n, b) ----
            x_t = {}
            xn_t = {}
            for b in range(BPC):
                for k in range(KC):
                    xt = pp.tile([128, N], FP32R, name=f"x_{b}_{k}")
                    nc.sync.dma_start(
                        xt[:], x_ext[b, k * 128:(k + 1) * 128, :].bitcast(FP32R))
                    x_t[b, k] = xt
            for b in range(BPC):
                sqs = []
                for k in range(KC):
                    sq = rot.tile([128, N], FP32R, name="sq", tag="sq", bufs=2)
                    nc.scalar.square(sq[:], x_t[b, k][:])
                    sqs.append(sq)
                pss = [ps2.tile([1, 512], FP32, name="pss", tag="ps2t")
                       for _ in range(2)]
                for h in range(2):
                    for k in range(KC):
                        nc.tensor.matmul(
                            pss[h][:], ones_col[:],
                            sqs[k][:, h * 512:(h + 1) * 512],
                            start=(k == 0), stop=(k == KC - 1))
                vsq = rot.tile([1, N], FP32, name="vsq", tag="vsq", bufs=1)
                for h in range(2):
                    nc.vector.tensor_scalar(
                        vsq[:, h * 512:(h + 1) * 512], pss[h][:], 1e-24, None,
                        AL.max)
                nc.scalar.sqrt(vsq[:], vsq[:])
                invn = rot.tile([1, N], FP32R, name="invn", tag="invn", bufs=1)
                with nc.allow_low_precision(reason="fp32r matmul inputs"):
                    nc.vector.reciprocal(invn[:], vsq[:])
                for k in range(KC):
                    xn_t[b, k] = big8.tile([128, N], FP32R, name=f"xn_{b}_{k}",
                                           tag="big")
                for h in range(2):
                    pbc = ps4.tile([128, 512], FP32, name="pbc", tag="ps4t")
                    nc.tensor.matmul(
                        pbc[:], ones_row[0:1, 0:128],
                        invn[0:1, h * 512:(h + 1) * 512],
                        start=True, stop=True)
                    for k in range(KC):
                        nc.vector.tensor_tensor(
                            xn_t[b, k][:, h * 512:(h + 1) * 512],
                            x_t[b, k][:, h * 512:(h + 1) * 512],
                            pbc[:], AL.mult)

            # ---- phase B: G_partial = sum_b Xn_b Xn_b^T -> DRAM bounce ----
            s_bounce = dram.tile([N, N], FP32, name="s_bounce")
            for m in range(MC):
                psS = [ps4.tile([128, 512], FP32, name="psS", tag="ps4t")
                       for _ in range(NF)]
                first = True
                for b in range(BPC):
                    for k in range(KC):
                        lhsT = xn_t[b, k][:, m * 128:(m + 1) * 128]
                        last = (b == BPC - 1 and k == KC - 1)
                        for h in range(NF):
                            nc.tensor.matmul(
                                psS[h][:], lhsT,
                                xn_t[b, k][:, h * 512:(h + 1) * 512],
                                start=first, stop=last)
                        first = False
                for h in range(NF):
                    sev = rot.tile([128, 512], FP32, name="sev", tag="sev")
                    nc.scalar.copy(sev[:], psS[h][:])
                    nc.sync.dma_start(
                        s_bounce[m * 128:(m + 1) * 128, h * 512:(h + 1) * 512],
                        sev[:])

            # ---- phase C: ReduceScatter -> owned 128 rows of G ----
            s_rs = dram.tile([ROWS, N], FP32, name="s_rs")
            nc.gpsimd.collective_compute(
                "ReduceScatter", AL.add,
                replica_groups=[list(range(NC))],
                ins=[s_bounce.opt()], outs=[s_rs.opt()])
            S_own = pp.tile([ROWS, N], FP32, name="S_own")
            nc.sync.dma_start(S_own[:], s_rs[:])

            # ---- phase D: xw_b = X_b @ W (overlaps the ReduceScatter) ----
            xw_t = {}
            for b in range(BPC):
                for m in range(MC):
                    pxw = ps4.tile([128, L], FP32, name="pxw", tag="ps4t")
                    for k in range(KC):
                        nc.tensor.matmul(
                            pxw[:], x_t[b, k][:, m * 128:(m + 1) * 128],
                            w_sb[k][:],
                            start=(k == 0), stop=(k == KC - 1))
                    xw = pp.tile([128, L], FP32R, name=f"xw_{b}_{m}")
                    nc.scalar.copy(xw[:], pxw[:])
                    xw_t[b, m] = xw

            # ---- phase E: per-row dyadic bisection for top-KSEL threshold ----
            tthr = pp.tile([128, 1], FP32, name="tthr")
            nc.vector.memset(tthr[:], -33.0)
            cnt = pp.tile([128, 1], FP32, name="cnt")
            probe = pp.tile([128, 1], FP32, name="probe")
            junk = iof
            step = 33.0
            for _ in range(NITER):
                # cnt[p] = #(G[p,:] >= tthr[p] + step)
                nc.vector.tensor_scalar(probe[:], tthr[:], step, None, AL.add)
                nc.vector.tensor_scalar(
                    junk[:], S_own[:], probe[:], 0.0, AL.is_ge, AL.add,
                    accum_out=cnt[:])
                nc.vector.tensor_scalar(cnt[:], cnt[:], float(KSEL), None,
                                        AL.is_ge)
                nc.vector.scalar_tensor_tensor(
                    tthr[:], cnt[:], step, tthr[:], AL.mult, AL.add)
                step *= 0.5

            # ---- phase F: masked A rows, AllGather full A ----
            A_own = pp.tile([ROWS, N], FP32, name="A_own")
            nc.vector.scalar_tensor_tensor(
                A_own[:], S_own[:], tthr[:], S_own[:], AL.is_ge, AL.mult)
            nc.vector.tensor_tensor(A_own[:], A_own[:], selfm[:], AL.mult)
            a_bounce = dram.tile([ROWS, N], FP32, name="a_bounce")
            nc.sync.dma_start(a_bounce[:], A_own[:])
            a_full = dram.tile([N, N], FP32, name="a_full", addr_space="Shared")
            nc.gpsimd.collective_compute(
                "AllGather", AL.bypass,
                replica_groups=[list(range(NC))],
                ins=[a_bounce.opt()], outs=[a_full.opt()])
            A_t = []
            for i in range(MC):
                at = big8.tile([128, N], FP32R, name=f"A_t{i}", tag="big")
                nc.sync.dma_start(at[:],
                                  a_full[i * 128:(i + 1) * 128, :].bitcast(FP32R))
                A_t.append(at)

            # ---- phase G: deg (column sums), dinv, An = dinv_s * A * dinv_d ----
            psd = [ps2.tile([1, 512], FP32, name="psd", tag="ps2t")
                   for _ in range(2)]
            for h in range(2):
                for i in range(MC):
                    nc.tensor.matmul(
                        psd[h][:], ones_col[:],
                        A_t[i][:, h * 512:(h + 1) * 512],
                        start=(i == 0), stop=(i == MC - 1))
            dgz = pp.tile([1, N], FP32, name="dgz")
            dmx = pp.tile([1, N], FP32, name="dmx")
            for h in range(2):
                nc.vector.tensor_scalar(
                    dgz[:, h * 512:(h + 1) * 512], psd[h][:], 0.0, None,
                    AL.is_gt)
                nc.vector.tensor_scalar(
                    dmx[:, h * 512:(h + 1) * 512], psd[h][:], 1e-30, None,
                    AL.max)
            nc.scalar.sqrt(dmx[:], dmx[:])
            rcp = pp.tile([1, N], FP32, name="rcp")
            nc.vector.reciprocal(rcp[:], dmx[:])
            dinv_f = pp.tile([1, N], FP32, name="dinv_f")
            nc.vector.tensor_tensor(dinv_f[:], rcp[:], dgz[:], AL.mult)
            dinv = pp.tile([1, N], FP32R, name="dinv")
            nc.vector.tensor_copy(dinv[:], dinv_f[:])
            onef_t = pp.tile([1, 1], FP32, name="onef_t")
            nc.vector.memset(onef_t[:], 1.0)
            # transpose dinv chunks into per-partition scalars drt[:, i]
            drt = pp.tile([128, MC], FP32, name="drt")
            pst = ps4.tile([128, MC], FP32, name="pst", tag="ps4t")
            for i in range(MC):
                nc.tensor.transpose(
                    pst[:, i:i + 1], dinv_f[0:1, i * 128:(i + 1) * 128],
                    onef_t[:])
            nc.scalar.copy(drt[:], pst[:])
            # broadcast dinv along partitions
            bc_sb = pp.tile([128, N], FP32, name="bc_sb")
            for h in range(2):
                pbc2 = ps4.tile([128, 512], FP32, name="pbc2", tag="ps4t")
                nc.tensor.matmul(
                    pbc2[:], ones_row[0:1, 0:128],
                    dinv[0:1, h * 512:(h + 1) * 512],
                    start=True, stop=True)
                nc.scalar.copy(bc_sb[:, h * 512:(h + 1) * 512], pbc2[:])
            for i in range(MC):
                nc.vector.scalar_tensor_tensor(
                    A_t[i][:], A_t[i][:], drt[:, i:i + 1], bc_sb[:],
                    AL.mult, AL.mult)

            # ---- phase H: out^T_b[l, d] = bias[l] + sum_s xw_b[s,l] An[s,d] ----
            for b in range(BPC):
                for lc in range(KC):
                    pso = [ps4.tile([128, 512], FP32, name="pso", tag="ps4t")
                           for _ in range(NF)]
                    for h in range(NF):
                        nc.tensor.matmul(
                            pso[h][:], bias_sb[0:1, lc * 128:(lc + 1) * 128],
                            ones_row[0:1, 0:512], start=True, stop=False)
                    for i in range(MC):
                        lhsT = xw_t[b, i][:, lc * 128:(lc + 1) * 128]
                        for h in range(NF):
                            nc.tensor.matmul(
                                pso[h][:], lhsT,
                                A_t[i][:, h * 512:(h + 1) * 512],
                                start=False, stop=(i == MC - 1))
                    for h in range(NF):
                        oev = rot.tile([128, 512], FP32, name="oev", tag="oev",
                                       bufs=4)
                        nc.scalar.copy(oev[:], pso[h][:])
                        nc.sync.dma_start(
                            o_ext[b, lc * 128:(lc + 1) * 128,
                                  h * 512:(h + 1) * 512],
                            oev[:])
    nc.compile()
    return nc


def get_nc():
    if "nc" not in _CACHE:
        _CACHE["nc"] = _build()
    return _CACHE["nc"]


def make_in_maps(x, weight, bias):
    x = np.ascontiguousarray(x, dtype=np.float32)
    w = np.ascontiguousarray(weight, dtype=np.float32)
    bias2 = np.ascontiguousarray(bias, dtype=np.float32).reshape(1, L)
    in_maps = []
    for c in range(NC):
        in_maps.append({
            "x": np.ascontiguousarray(x[c * BPC:(c + 1) * BPC]),
            "w": w,
            "bias": bias2,
            "ridx": (np.arange(128, dtype=np.float32)[:, None] + c * ROWS),
        })
    return in_maps


def kernel(x, weight, bias, _trace=False):
    nc = get_nc()
    in_maps = make_in_maps(x, weight, bias)
    res = run_bass_kernel_spmd(nc, in_maps, list(range(NC)), trace=_trace)
    out = np.concatenate([res.results[c]["out"] for c in range(NC)], axis=0)
    if _trace:
        _CACHE["last_exec_time_ns"] = res.exec_time_ns
    return out
